# revision 28
# baseline (speedup 1.0000x reference)
"""MultiHeadAttention block (LN -> QKV -> attention -> out-proj + residual,
plus mean-over-heads attention output) on 8 TRN2 NeuronCores.

Sharding: tensor-parallel over heads — 16 heads / 8 cores = 2 heads per core.
Each core computes LN (replicated), its 2 heads' Q/K/V projections, attention,
an attn-mean partial (sum of its 2 normalized heads / 16), and a partial of
the output projection (Wo input-dim sharded). Host sums partials.

Device layout notes (per core):
  - Everything feature-major: z^T [D, token], Q^T/K^T/V^T [128, 4096] where
    rows 0-63 = even head, 64-127 = odd head; cols = b*2048 + s.
  - Scores computed directly transposed: S^T[k, q] = (K^T slice).T @ (Q^T slice)
    so exp(S^T) = P^T feeds P@V as the moving operand with no transpose of P.
  - V-hat = [V | 1] so the P@V matmul's row 64 accumulates the softmax sums.
  - Context arrives as C_raw^T [65, q]; a small transpose dance moves it to
    q-partition layout, scales rows by r = 1/sum, and transposes back for the
    output projection.
"""

import os

import numpy as np
import ml_dtypes

import concourse.bass as bass
import concourse.mybir as mybir
import concourse.tile as tile
from concourse import bacc
from concourse.bass_utils import run_bass_kernel_spmd
from concourse.masks import make_identity

F32 = mybir.dt.float32
F32R = mybir.dt.float32r
BF16 = mybir.dt.bfloat16
AF = mybir.ActivationFunctionType
OP = mybir.AluOpType

B, S, D = 2, 2048, 1024
H, DK = 16, 64
T = B * S            # 4096 tokens
NCORES = 8
HPC = 2              # heads per core
QB = 1024            # q-block
NKT = S // 128       # 16 k-tiles per batch
NTG = T // 512       # 8 token groups
EPS = 1e-5

_NC_CACHE = {}


def _build_core_program():
    nc = bacc.Bacc(None)

    x_d = nc.dram_tensor("x", [T, D], F32, kind="ExternalInput")
    w_d = nc.dram_tensor("wqkv", [D, 3, 128], BF16, kind="ExternalInput")
    b_d = nc.dram_tensor("bqkv", [128, 3], F32, kind="ExternalInput")
    wo_d = nc.dram_tensor("wo", [HPC, DK, D], F32R, kind="ExternalInput")
    attn_d = nc.dram_tensor("attn_t", [B, S // QB, NKT, 128, QB], BF16, kind="ExternalOutput")
    out1_d = nc.dram_tensor("out1_t", [8, T // 512, 128, 512], BF16, kind="ExternalOutput")
    DBG = bool(int(os.environ.get("BASSDBG", "0")))
    if DBG:
        qt_d = nc.dram_tensor("dbg_qt", [128, T], BF16, kind="ExternalOutput")
        kt_d = nc.dram_tensor("dbg_kt", [128, T], BF16, kind="ExternalOutput")
        vt_d = nc.dram_tensor("dbg_vt", [128, T], BF16, kind="ExternalOutput")
        cr_d = nc.dram_tensor("dbg_craw", [2, 65, QB], F32, kind="ExternalOutput")
        pt_d = nc.dram_tensor("dbg_pt", [128, QB], BF16, kind="ExternalOutput")
        pt2_d = nc.dram_tensor("dbg_pt2", [128, QB], BF16, kind="ExternalOutput")
        pt3_d = nc.dram_tensor("dbg_pt3", [128, QB], BF16, kind="ExternalOutput")
        vh_d = nc.dram_tensor("dbg_vhat", [128, B * HPC * NKT, 80], BF16, kind="ExternalOutput")

    from contextlib import ExitStack

    with ExitStack() as ctx:
        tc = ctx.enter_context(tile.TileContext(nc))
        constp = ctx.enter_context(tc.tile_pool(name="const", bufs=1))
        persist = ctx.enter_context(tc.tile_pool(name="persist", bufs=1))
        smallp = ctx.enter_context(tc.tile_pool(name="smallp", bufs=4))
        phase1 = ExitStack()
        xp = phase1.enter_context(tc.tile_pool(name="xp", bufs=4))
        zp = phase1.enter_context(tc.tile_pool(name="zp", bufs=6))
        ztp = phase1.enter_context(tc.tile_pool(name="ztp", bufs=16))
        ps_zt = phase1.enter_context(tc.tile_pool(name="ps_zt", bufs=4, space="PSUM"))
        ps_qkv = phase1.enter_context(tc.tile_pool(name="ps_qkv", bufs=3, space="PSUM"))
        if True:
            # ---- constants / weights ----
            ident = constp.tile([128, 128], F32, name="ident")
            make_identity(nc, ident[:])
            ident_r = constp.tile([128, 128], F32R, name="ident_r")
            nc.vector.tensor_copy(ident_r[:], ident[:])
            eps_t = constp.tile([128, 1], F32, name="eps_t")
            nc.gpsimd.memset(eps_t[:], EPS)

            w_sb = persist.tile([128, 8, 3, 128], BF16, name="w_sb")
            nc.sync.dma_start(
                w_sb[:], w_d.ap().rearrange("(dc p) k o -> p dc k o", p=128)
            )
            bias_sb = persist.tile([128, 3], F32, name="bias_sb")
            nc.sync.dma_start(bias_sb[:], b_d[:])
            wo_sb = persist.tile([DK, HPC, 8, 128], F32R, name="wo_sb")
            nc.sync.dma_start(
                wo_sb[:], wo_d.ap().rearrange("h p (c f) -> p h c f", f=128)
            )

            QT = persist.tile([128, T], BF16, name="QT")
            KT = persist.tile([128, T], BF16, name="KT")
            VT = persist.tile([128, T], BF16, name="VT")
            vhat = persist.tile([128, B * HPC * NKT, 80], BF16, name="vhat")

            # ---- Phase A/B: LayerNorm + transposes + QKV projections ----
            for tg in range(NTG):
                z_ts = []
                zts = []
                for dc in range(8):
                    zt = ztp.tile([128, 512], BF16, tag="zt", name=f"zt_{tg}_{dc}")
                    zts.append(zt)
                for i in range(4):
                    x_t = xp.tile([128, D], F32, tag="x", name=f"x_{tg}_{i}")
                    nc.sync.dma_start(x_t[:], x_d[(tg * 4 + i) * 128:(tg * 4 + i + 1) * 128, :])
                    stats = smallp.tile([128, 2, 6], F32, tag="stats", name=f"st_{tg}_{i}")
                    xv = x_t[:].rearrange("p (g f) -> p g f", f=512)
                    nc.vector.bn_stats(stats[:, 0, :], xv[:, 0, :])
                    nc.vector.bn_stats(stats[:, 1, :], xv[:, 1, :])
                    mv = smallp.tile([128, 2], F32, tag="mv", name=f"mv_{tg}_{i}")
                    nc.vector.bn_aggr(mv[:], stats[:])
                    sd = smallp.tile([128, 1], F32, tag="sd", name=f"sd_{tg}_{i}")
                    nc.scalar.activation(sd[:], mv[:, 1:2], AF.Sqrt, bias=eps_t[:])
                    rstd = smallp.tile([128, 1], F32, tag="rstd", name=f"rs_{tg}_{i}")
                    nc.vector.reciprocal(rstd[:], sd[:])
                    nmr = smallp.tile([128, 1], F32, tag="nmr", name=f"nm_{tg}_{i}")
                    nc.vector.tensor_scalar(
                        nmr[:], mv[:, 0:1], rstd[:], -1.0, OP.mult, OP.mult
                    )
                    z_t = zp.tile([128, D], F32, tag="z", name=f"z_{tg}_{i}")
                    nc.gpsimd.tensor_scalar(
                        z_t[:], x_t[:], rstd[:], nmr[:], OP.mult, OP.add
                    )
                    z_ts.append(z_t)
                for dc in range(8):
                    zps = ps_zt.tile([128, 512], F32, tag="zt", name=f"zps_{tg}_{dc}")
                    for i in range(4):
                        nc.tensor.transpose(
                            zps[:, i * 128:(i + 1) * 128],
                            z_ts[i][:, dc * 128:(dc + 1) * 128],
                            ident[:],
                        )
                    nc.any.tensor_copy(zts[dc][:], zps[:])
                for p3, dst in ((0, QT), (1, KT), (2, VT)):
                    pq = ps_qkv.tile([128, 512], F32, tag="qkv", name=f"pq_{tg}_{p3}")
                    for dc in range(8):
                        nc.tensor.matmul(
                            pq[:], w_sb[:, dc, p3, :], zts[dc][:],
                            start=(dc == 0), stop=(dc == 7),
                        )
                    nc.vector.tensor_scalar(
                        dst[:, tg * 512:(tg + 1) * 512], pq[:],
                        bias_sb[:, p3:p3 + 1], None, OP.add,
                    )

            if DBG:
                nc.sync.dma_start(qt_d.ap(), QT[:])
                nc.sync.dma_start(kt_d.ap(), KT[:])
                nc.sync.dma_start(vt_d.ap(), VT[:])

            # ---- Phase V: build V-hat = [V | 1] per (b, head, ktile) ----
            nc.vector.memset(vhat[:, :, 64:65], 1.0)
            for b in range(B):
                for h2 in range(HPC):
                    for kt in range(NKT):
                        vidx = (b * HPC + h2) * NKT + kt
                        nc.scalar.dma_start_transpose(
                            vhat[:, vidx, 0:64],
                            VT[h2 * 64:(h2 + 1) * 64,
                               b * S + kt * 128:b * S + (kt + 1) * 128],
                        )

            phase1.close()
            ptp = ctx.enter_context(tc.tile_pool(name="ptp", bufs=44))
            crawp = ctx.enter_context(tc.tile_pool(name="crawp", bufs=2))
            ctp = ctx.enter_context(tc.tile_pool(name="ctp", bufs=2))
            apool = ctx.enter_context(tc.tile_pool(name="ap", bufs=3))
            rbp = ctx.enter_context(tc.tile_pool(name="rbp", bufs=4))
            opool = ctx.enter_context(tc.tile_pool(name="op", bufs=2))
            r1p = ctx.enter_context(tc.tile_pool(name="r1p", bufs=2))
            ps_mix = ctx.enter_context(tc.tile_pool(name="ps_mix", bufs=2, space="PSUM"))
            ps_s = ctx.enter_context(tc.tile_pool(name="ps_s", bufs=2, space="PSUM"))
            ps_d = ctx.enter_context(tc.tile_pool(name="ps_d", bufs=2, space="PSUM"))

            # ---- Phase C: attention, software-pipelined per (batch, q-block):
            # head (scores/exp/PV) of block n+1 is emitted before the tail
            # (normalize dance, attn-combine, out-proj) of block n.
            def attn_head(b, qb):
                    q0 = b * S + qb * QB
                    pts = {}
                    for kt in range(NKT):
                        for h2 in range(HPC):
                            pss = ps_s.tile([128, QB], F32, tag="s", name=f"s_{b}_{qb}_{kt}_{h2}")
                            lhs = KT[h2 * 64:(h2 + 1) * 64,
                                     b * S + kt * 128:b * S + (kt + 1) * 128]
                            for j in range(QB // 512):
                                nc.tensor.matmul(
                                    pss[:, j * 512:(j + 1) * 512], lhs,
                                    QT[h2 * 64:(h2 + 1) * 64,
                                       q0 + j * 512:q0 + (j + 1) * 512],
                                    start=True, stop=True,
                                )
                            pt = ptp.tile([128, QB], BF16, tag="pt", name=f"pt_{b}_{qb}_{kt}_{h2}")
                            nc.scalar.activation(pt[:], pss[:], AF.Exp, scale=0.125)
                            pts[(kt, h2)] = pt
                    # P @ V-hat, accumulating over k tiles; row 64 = softmax sums
                    craw = crawp.tile([65, QB], F32, tag="craw", name=f"cr_{b}_{qb}")
                    craw2 = crawp.tile([65, QB], F32, tag="craw2", name=f"cr2_{b}_{qb}")
                    for qh in range(QB // 512):
                        for h2 in range(HPC):
                            pcc = ps_mix.tile([128, 512], F32, tag="mix", name=f"c_{b}_{qb}_{qh}_{h2}")
                            for kt in range(NKT):
                                vidx = (b * HPC + h2) * NKT + kt
                                nc.tensor.matmul(
                                    pcc[:65, :], vhat[:, vidx, 0:65],
                                    pts[(kt, h2)][:, qh * 512:(qh + 1) * 512],
                                    start=(kt == 0), stop=(kt == NKT - 1),
                                )
                            dst_craw = craw if h2 == 0 else craw2
                            nc.scalar.copy(
                                dst_craw[:, qh * 512:(qh + 1) * 512], pcc[:65, :]
                            )
                    if DBG and b == 0 and qb == 0:
                        nc.sync.dma_start(cr_d[0], craw[:])
                        nc.sync.dma_start(cr_d[1], craw2[:])
                        nc.sync.dma_start(pt_d.ap(), pts[(0, 0)][:])
                        nc.sync.dma_start(pt2_d.ap(), pts[(15, 0)][:])
                        nc.sync.dma_start(pt3_d.ap(), pts[(0, 1)][:])
                        nc.sync.dma_start(vh_d.ap(), vhat[:])
                    return pts, craw, craw2

            def attn_tail(b, qb, pts, craw, craw2):
                    q0 = b * S + qb * QB
                    # dance: [65, q] -> q-partition -> scale by 1/sum -> back
                    CT = ctp.tile([DK, HPC, QB], F32R, tag="ct", name=f"ct_{b}_{qb}")
                    r16c = smallp.tile([128, 16], F32, tag="r16c", name=f"r16_{b}_{qb}")
                    for h2 in range(HPC):
                        src = craw if h2 == 0 else craw2
                        for ch in range(QB // 128):
                            t1 = ps_d.tile([128, 130], F32, tag="d", name=f"t1_{b}_{qb}_{h2}_{ch}")
                            nc.tensor.transpose(
                                t1[:, 0:65], src[:, ch * 128:(ch + 1) * 128],
                                ident[0:65, 0:65],
                            )
                            rec = smallp.tile([128, 1], F32, tag="rec", name=f"re_{b}_{qb}_{h2}_{ch}")
                            nc.vector.reciprocal(rec[:], t1[:, 64:65])
                            nc.vector.tensor_scalar(
                                r16c[:, h2 * 8 + ch:h2 * 8 + ch + 1], rec[:],
                                1.0 / H, None, OP.mult,
                            )
                            cn = smallp.tile([128, 64], F32R, tag="cn", name=f"cn_{b}_{qb}_{h2}_{ch}")
                            nc.vector.tensor_scalar(
                                cn[:], t1[:, 0:64], rec[:], None, OP.mult
                            )
                            t2 = ps_d.tile([64, 128], F32R, tag="d", name=f"t2_{b}_{qb}_{h2}_{ch}")
                            nc.tensor.transpose(t2[:], cn[:], ident_r[:])
                            nc.vector.tensor_copy(
                                CT[:, h2, ch * 128:(ch + 1) * 128], t2[:]
                            )
                    # r16 columns -> row form -> broadcast tiles
                    rtp = ps_d.tile([16, 128], F32, tag="d", name=f"rt_{b}_{qb}")
                    nc.tensor.transpose(rtp[:], r16c[:], ident[:])
                    rrow = smallp.tile([16, 128], BF16, tag="rrow", name=f"rr_{b}_{qb}")
                    nc.vector.tensor_copy(rrow[:], rtp[:])
                    rbs = []
                    for h2 in range(HPC):
                        r1 = r1p.tile([1, QB], BF16, tag="r1", name=f"r1_{b}_{qb}_{h2}")
                        nc.sync.dma_start(
                            r1[:], rrow[h2 * 8:(h2 + 1) * 8, :]
                        )
                        rb = rbp.tile([128, QB], BF16, tag="rb", name=f"rb_{b}_{qb}_{h2}")
                        nc.gpsimd.partition_broadcast(rb[:], r1[:])
                        rbs.append(rb)
                    # attn-mean partial: (P0*r0 + P1*r1), bf16 out
                    for kt in range(NKT):
                        a0 = apool.tile([128, QB], BF16, tag="a0", name=f"a0_{b}_{qb}_{kt}")
                        nc.vector.tensor_tensor(a0[:], pts[(kt, 0)][:], rbs[0][:], OP.mult)
                        a1 = apool.tile([128, QB], BF16, tag="a1", name=f"a1_{b}_{qb}_{kt}")
                        nc.gpsimd.tensor_tensor(a1[:], pts[(kt, 1)][:], rbs[1][:], OP.mult)
                        nc.vector.tensor_tensor(a0[:], a0[:], a1[:], OP.add)
                        nc.sync.dma_start(attn_d[b, qb, kt], a0[:])
                    # output projection partial for this (b, qb)
                    for tch in range(QB // 512):
                        for do in range(8):
                            po = ps_d.tile([128, 512], F32, tag="d", name=f"po_{b}_{qb}_{tch}_{do}")
                            for h2 in range(HPC):
                                nc.tensor.matmul(
                                    po[:], wo_sb[:, h2, do, :],
                                    CT[:, h2, tch * 512:(tch + 1) * 512],
                                    start=(h2 == 0), stop=(h2 == HPC - 1),
                                )
                            ob = opool.tile([128, 512], BF16, tag="ob", name=f"ob_{b}_{qb}_{tch}_{do}")
                            nc.any.tensor_copy(ob[:], po[:])
                            nc.sync.dma_start(
                                out1_d[do, (q0 + tch * 512) // 512], ob[:]
                            )

            blocks = [(b, qb) for b in range(B) for qb in range(S // QB)]
            prev = None
            for b, qb in blocks:
                head = attn_head(b, qb)
                if prev is not None:
                    attn_tail(*prev)
                prev = (b, qb, *head)
            attn_tail(*prev)
    nc.compile()
    return nc


def _get_nc():
    if "nc" not in _NC_CACHE:
        _NC_CACHE["nc"] = _build_core_program()
    return _NC_CACHE["nc"]


def kernel(x, Wq, bq, Wk, bk, Wv, bv, Wo, bo, ln_gamma, ln_beta):
    x = np.asarray(x, np.float32)
    Wq, bq = np.asarray(Wq, np.float32), np.asarray(bq, np.float32)
    Wk, bk = np.asarray(Wk, np.float32), np.asarray(bk, np.float32)
    Wv, bv = np.asarray(Wv, np.float32), np.asarray(bv, np.float32)
    Wo, bo = np.asarray(Wo, np.float32), np.asarray(bo, np.float32)
    g = np.asarray(ln_gamma, np.float32)
    be = np.asarray(ln_beta, np.float32)

    x2 = np.ascontiguousarray(x.reshape(T, D))

    in_maps = []
    for c in range(NCORES):
        sl = slice(128 * c, 128 * (c + 1))
        # fold LN gamma into W, LN beta into bias: proj = z @ (W*g).T + (W@be + b)
        wstk = np.stack(
            [
                (Wq * g[None, :])[sl, :],
                (Wk * g[None, :])[sl, :],
                (Wv * g[None, :])[sl, :],
            ],
            axis=0,
        )  # [3, 128, D]
        bstk = np.stack(
            [
                (Wq @ be + bq)[sl],
                (Wk @ be + bk)[sl],
                (Wv @ be + bv)[sl],
            ],
            axis=1,
        ).astype(np.float32)  # [128, 3]
        w_c = np.ascontiguousarray(
            wstk.transpose(2, 0, 1).astype(ml_dtypes.bfloat16)
        )  # [D, 3, 128]
        wo_c = np.ascontiguousarray(
            Wo[:, sl].T.reshape(HPC, DK, D).astype(np.float32)
        )  # [2, 64, D]
        in_maps.append({"x": x2, "wqkv": w_c, "bqkv": bstk, "wo": wo_c})

    nc = _get_nc()
    res = run_bass_kernel_spmd(nc, in_maps, core_ids=list(range(NCORES)))

    out1_acc = np.zeros((8, T // 512, 128, 512), np.float32)
    attn_acc = np.zeros((B, S // QB, NKT, 128, QB), np.float32)
    for r in res.results:
        out1_acc += r["out1_t"].astype(np.float32)
        attn_acc += r["attn_t"].astype(np.float32)

    out1 = out1_acc.transpose(0, 2, 1, 3).reshape(D, T).T  # [T, D]
    out1 = out1 + bo[None, :] + x2
    out = out1.reshape(B, S, D)
    # attn_acc[b, qb, kt, p, j] = A^T[k=kt*128+p, q=qb*QB+j]
    attn = np.ascontiguousarray(
        attn_acc.transpose(0, 1, 4, 2, 3).reshape(B, S, S)
    )
    return out, attn


# revision 29
# speedup vs baseline: 1.0898x; 1.0898x over previous
"""MultiHeadAttention block (LN -> QKV -> attention -> out-proj + residual,
plus mean-over-heads attention output) on 8 TRN2 NeuronCores.

Sharding: tensor-parallel over heads — 16 heads / 8 cores = 2 heads per core.
Each core computes LN (replicated), its 2 heads' Q/K/V projections, attention,
an attn-mean partial (sum of its 2 normalized heads / 16), and a partial of
the output projection (Wo input-dim sharded). Host sums partials.

Device layout notes (per core):
  - Everything feature-major: z^T [D, token], Q^T/K^T/V^T [128, 4096] where
    rows 0-63 = even head, 64-127 = odd head; cols = b*2048 + s.
  - Scores computed directly transposed: S^T[k, q] = (K^T slice).T @ (Q^T slice)
    so exp(S^T) = P^T feeds P@V as the moving operand with no transpose of P.
  - V-hat = [V | 1] so the P@V matmul's row 64 accumulates the softmax sums.
  - Context arrives as C_raw^T [65, q]; a small transpose dance moves it to
    q-partition layout, scales rows by r = 1/sum, and transposes back for the
    output projection.
"""

import os

import numpy as np
import ml_dtypes

import concourse.bass as bass
import concourse.mybir as mybir
import concourse.tile as tile
from concourse import bacc
from concourse.bass_utils import run_bass_kernel_spmd
from concourse.masks import make_identity

F32 = mybir.dt.float32
F32R = mybir.dt.float32r
BF16 = mybir.dt.bfloat16
AF = mybir.ActivationFunctionType
OP = mybir.AluOpType

B, S, D = 2, 2048, 1024
H, DK = 16, 64
T = B * S            # 4096 tokens
NCORES = 8
HPC = 2              # heads per core
QB = 1024            # q-block
NKT = S // 128       # 16 k-tiles per batch
NTG = T // 512       # 8 token groups
EPS = 1e-5

_NC_CACHE = {}


def _build_core_program():
    nc = bacc.Bacc(None)

    x_d = nc.dram_tensor("x", [T, D], F32, kind="ExternalInput")
    w_d = nc.dram_tensor("wqkv", [D, 3, 128], BF16, kind="ExternalInput")
    b_d = nc.dram_tensor("bqkv", [128, 3], F32, kind="ExternalInput")
    wo_d = nc.dram_tensor("wo", [HPC, DK, D], F32R, kind="ExternalInput")
    attn_d = nc.dram_tensor("attn_t", [B, S // QB, NKT, 128, QB], BF16, kind="ExternalOutput")
    out1_d = nc.dram_tensor("out1_t", [8, T // 512, 128, 512], BF16, kind="ExternalOutput")
    DBG = bool(int(os.environ.get("BASSDBG", "0")))
    if DBG:
        qt_d = nc.dram_tensor("dbg_qt", [128, T], BF16, kind="ExternalOutput")
        kt_d = nc.dram_tensor("dbg_kt", [128, T], BF16, kind="ExternalOutput")
        vt_d = nc.dram_tensor("dbg_vt", [128, T], BF16, kind="ExternalOutput")
        cr_d = nc.dram_tensor("dbg_craw", [2, 65, QB], F32, kind="ExternalOutput")
        pt_d = nc.dram_tensor("dbg_pt", [128, QB], BF16, kind="ExternalOutput")
        pt2_d = nc.dram_tensor("dbg_pt2", [128, QB], BF16, kind="ExternalOutput")
        pt3_d = nc.dram_tensor("dbg_pt3", [128, QB], BF16, kind="ExternalOutput")
        vh_d = nc.dram_tensor("dbg_vhat", [128, B * HPC * NKT, 80], BF16, kind="ExternalOutput")

    from contextlib import ExitStack

    with ExitStack() as ctx:
        tc = ctx.enter_context(tile.TileContext(nc))
        constp = ctx.enter_context(tc.tile_pool(name="const", bufs=1))
        persist = ctx.enter_context(tc.tile_pool(name="persist", bufs=1))
        smallp = ctx.enter_context(tc.tile_pool(name="smallp", bufs=4))
        phase1 = ExitStack()
        xp = phase1.enter_context(tc.tile_pool(name="xp", bufs=4))
        zp = phase1.enter_context(tc.tile_pool(name="zp", bufs=6))
        ztp = phase1.enter_context(tc.tile_pool(name="ztp", bufs=16))
        ps_zt = phase1.enter_context(tc.tile_pool(name="ps_zt", bufs=4, space="PSUM"))
        ps_qkv = phase1.enter_context(tc.tile_pool(name="ps_qkv", bufs=3, space="PSUM"))
        if True:
            # ---- constants / weights ----
            ident = constp.tile([128, 128], F32, name="ident")
            make_identity(nc, ident[:])
            ident_r = constp.tile([128, 128], F32R, name="ident_r")
            nc.vector.tensor_copy(ident_r[:], ident[:])
            eps_t = constp.tile([128, 1], F32, name="eps_t")
            nc.gpsimd.memset(eps_t[:], EPS)

            w_sb = persist.tile([128, 8, 3, 128], BF16, name="w_sb")
            nc.sync.dma_start(
                w_sb[:], w_d.ap().rearrange("(dc p) k o -> p dc k o", p=128)
            )
            bias_sb = persist.tile([128, 3], F32, name="bias_sb")
            nc.sync.dma_start(bias_sb[:], b_d[:])
            wo_sb = persist.tile([DK, HPC, 8, 128], F32R, name="wo_sb")
            nc.sync.dma_start(
                wo_sb[:], wo_d.ap().rearrange("h p (c f) -> p h c f", f=128)
            )

            QT = persist.tile([128, T], BF16, name="QT")
            KT = persist.tile([128, T], BF16, name="KT")
            VT = persist.tile([128, T], BF16, name="VT")
            vhat = persist.tile([128, B * HPC * NKT, 80], BF16, name="vhat")

            # ---- Phase A/B: LayerNorm + transposes + QKV projections ----
            for tg in range(NTG):
                z_ts = []
                zts = []
                for dc in range(8):
                    zt = ztp.tile([128, 512], BF16, tag="zt", name=f"zt_{tg}_{dc}")
                    zts.append(zt)
                for i in range(4):
                    x_t = xp.tile([128, D], F32, tag="x", name=f"x_{tg}_{i}")
                    nc.sync.dma_start(x_t[:], x_d[(tg * 4 + i) * 128:(tg * 4 + i + 1) * 128, :])
                    stats = smallp.tile([128, 2, 6], F32, tag="stats", name=f"st_{tg}_{i}")
                    xv = x_t[:].rearrange("p (g f) -> p g f", f=512)
                    nc.vector.bn_stats(stats[:, 0, :], xv[:, 0, :])
                    nc.vector.bn_stats(stats[:, 1, :], xv[:, 1, :])
                    mv = smallp.tile([128, 2], F32, tag="mv", name=f"mv_{tg}_{i}")
                    nc.vector.bn_aggr(mv[:], stats[:])
                    sd = smallp.tile([128, 1], F32, tag="sd", name=f"sd_{tg}_{i}")
                    nc.scalar.activation(sd[:], mv[:, 1:2], AF.Sqrt, bias=eps_t[:])
                    rstd = smallp.tile([128, 1], F32, tag="rstd", name=f"rs_{tg}_{i}")
                    nc.vector.reciprocal(rstd[:], sd[:])
                    nmr = smallp.tile([128, 1], F32, tag="nmr", name=f"nm_{tg}_{i}")
                    nc.vector.tensor_scalar(
                        nmr[:], mv[:, 0:1], rstd[:], -1.0, OP.mult, OP.mult
                    )
                    z_t = zp.tile([128, D], F32, tag="z", name=f"z_{tg}_{i}")
                    nc.gpsimd.tensor_scalar(
                        z_t[:], x_t[:], rstd[:], nmr[:], OP.mult, OP.add
                    )
                    z_ts.append(z_t)
                for dc in range(8):
                    zps = ps_zt.tile([128, 512], F32, tag="zt", name=f"zps_{tg}_{dc}")
                    for i in range(4):
                        nc.tensor.transpose(
                            zps[:, i * 128:(i + 1) * 128],
                            z_ts[i][:, dc * 128:(dc + 1) * 128],
                            ident[:],
                        )
                    nc.any.tensor_copy(zts[dc][:], zps[:])
                for p3, dst in ((0, QT), (1, KT), (2, VT)):
                    pq = ps_qkv.tile([128, 512], F32, tag="qkv", name=f"pq_{tg}_{p3}")
                    for dc in range(8):
                        nc.tensor.matmul(
                            pq[:], w_sb[:, dc, p3, :], zts[dc][:],
                            start=(dc == 0), stop=(dc == 7),
                        )
                    nc.vector.tensor_scalar(
                        dst[:, tg * 512:(tg + 1) * 512], pq[:],
                        bias_sb[:, p3:p3 + 1], None, OP.add,
                    )

            if DBG:
                nc.sync.dma_start(qt_d.ap(), QT[:])
                nc.sync.dma_start(kt_d.ap(), KT[:])
                nc.sync.dma_start(vt_d.ap(), VT[:])

            # ---- Phase V: build V-hat = [V | 1] per (b, head, ktile) ----
            nc.vector.memset(vhat[:, :, 64:65], 1.0)
            for b in range(B):
                for h2 in range(HPC):
                    for kt in range(NKT):
                        vidx = (b * HPC + h2) * NKT + kt
                        nc.scalar.dma_start_transpose(
                            vhat[:, vidx, 0:64],
                            VT[h2 * 64:(h2 + 1) * 64,
                               b * S + kt * 128:b * S + (kt + 1) * 128],
                        )

            phase1.close()
            ptp = ctx.enter_context(tc.tile_pool(name="ptp", bufs=43))
            crawp = ctx.enter_context(tc.tile_pool(name="crawp", bufs=2))
            ctp = ctx.enter_context(tc.tile_pool(name="ctp", bufs=2))
            apool = ctx.enter_context(tc.tile_pool(name="ap", bufs=3))
            rbp = ctx.enter_context(tc.tile_pool(name="rbp", bufs=4))
            opool = ctx.enter_context(tc.tile_pool(name="op", bufs=2))
            r1p = ctx.enter_context(tc.tile_pool(name="r1p", bufs=1))
            cnp = ctx.enter_context(tc.tile_pool(name="cnp", bufs=18))
            ps_mix = ctx.enter_context(tc.tile_pool(name="ps_mix", bufs=2, space="PSUM"))
            ps_s = ctx.enter_context(tc.tile_pool(name="ps_s", bufs=2, space="PSUM"))
            ps_d = ctx.enter_context(tc.tile_pool(name="ps_d", bufs=2, space="PSUM"))

            # ---- Phase C: attention, software-pipelined per (batch, q-block):
            # head (scores/exp/PV) of block n+1 is emitted before the tail
            # (normalize dance, attn-combine, out-proj) of block n.
            def attn_head(b, qb):
                    q0 = b * S + qb * QB
                    pts = {}
                    for kt in range(NKT):
                        for h2 in range(HPC):
                            pss = ps_s.tile([128, QB], F32, tag="s", name=f"s_{b}_{qb}_{kt}_{h2}")
                            lhs = KT[h2 * 64:(h2 + 1) * 64,
                                     b * S + kt * 128:b * S + (kt + 1) * 128]
                            for j in range(QB // 512):
                                nc.tensor.matmul(
                                    pss[:, j * 512:(j + 1) * 512], lhs,
                                    QT[h2 * 64:(h2 + 1) * 64,
                                       q0 + j * 512:q0 + (j + 1) * 512],
                                    start=True, stop=True,
                                )
                            pt = ptp.tile([128, QB], BF16, tag="pt", name=f"pt_{b}_{qb}_{kt}_{h2}")
                            nc.scalar.activation(pt[:], pss[:], AF.Exp, scale=0.125)
                            pts[(kt, h2)] = pt
                    # P @ V-hat, accumulating over k tiles; row 64 = softmax sums
                    craw = crawp.tile([65, QB], F32, tag="craw", name=f"cr_{b}_{qb}")
                    craw2 = crawp.tile([65, QB], F32, tag="craw2", name=f"cr2_{b}_{qb}")
                    for qh in range(QB // 512):
                        for h2 in range(HPC):
                            pcc = ps_mix.tile([128, 512], F32, tag="mix", name=f"c_{b}_{qb}_{qh}_{h2}")
                            for kt in range(NKT):
                                vidx = (b * HPC + h2) * NKT + kt
                                nc.tensor.matmul(
                                    pcc[:65, :], vhat[:, vidx, 0:65],
                                    pts[(kt, h2)][:, qh * 512:(qh + 1) * 512],
                                    start=(kt == 0), stop=(kt == NKT - 1),
                                )
                            dst_craw = craw if h2 == 0 else craw2
                            nc.scalar.copy(
                                dst_craw[:, qh * 512:(qh + 1) * 512], pcc[:65, :]
                            )
                    if DBG and b == 0 and qb == 0:
                        nc.sync.dma_start(cr_d[0], craw[:])
                        nc.sync.dma_start(cr_d[1], craw2[:])
                        nc.sync.dma_start(pt_d.ap(), pts[(0, 0)][:])
                        nc.sync.dma_start(pt2_d.ap(), pts[(15, 0)][:])
                        nc.sync.dma_start(pt3_d.ap(), pts[(0, 1)][:])
                        nc.sync.dma_start(vh_d.ap(), vhat[:])
                    return pts, craw, craw2

            def attn_tail(b, qb, pts, craw, craw2):
                    q0 = b * S + qb * QB
                    # dance pass 1: transpose, extract 1/sum, normalize context.
                    # (transpose-back to CT is deferred so r16c completes early)
                    CT = ctp.tile([DK, HPC, QB], F32R, tag="ct", name=f"ct_{b}_{qb}")
                    r16c = smallp.tile([128, 16], F32, tag="r16c", name=f"r16_{b}_{qb}")
                    cns = {}
                    for h2 in range(HPC):
                        src = craw if h2 == 0 else craw2
                        for ch in range(QB // 128):
                            t1 = ps_d.tile([128, 130], F32, tag="d", name=f"t1_{b}_{qb}_{h2}_{ch}")
                            nc.tensor.transpose(
                                t1[:, 0:65], src[:, ch * 128:(ch + 1) * 128],
                                ident[0:65, 0:65],
                            )
                            rec = smallp.tile([128, 1], F32, tag="rec", name=f"re_{b}_{qb}_{h2}_{ch}")
                            nc.vector.reciprocal(rec[:], t1[:, 64:65])
                            nc.vector.tensor_scalar(
                                r16c[:, h2 * 8 + ch:h2 * 8 + ch + 1], rec[:],
                                1.0 / H, None, OP.mult,
                            )
                            cn = cnp.tile([128, 64], F32R, tag="cn", name=f"cn_{b}_{qb}_{h2}_{ch}")
                            nc.vector.tensor_scalar(
                                cn[:], t1[:, 0:64], rec[:], None, OP.mult
                            )
                            cns[(h2, ch)] = cn
                    # r16 columns -> row form -> broadcast tiles
                    rtp = ps_d.tile([16, 128], F32, tag="d", name=f"rt_{b}_{qb}")
                    nc.tensor.transpose(rtp[:], r16c[:], ident[:])
                    rrow = smallp.tile([16, 128], BF16, tag="rrow", name=f"rr_{b}_{qb}")
                    nc.vector.tensor_copy(rrow[:], rtp[:])
                    rbs = []
                    for h2 in range(HPC):
                        r1 = r1p.tile([1, QB], BF16, tag="r1", name=f"r1_{b}_{qb}_{h2}")
                        nc.sync.dma_start(
                            r1[:], rrow[h2 * 8:(h2 + 1) * 8, :]
                        )
                        rb = rbp.tile([128, QB], BF16, tag="rb", name=f"rb_{b}_{qb}_{h2}")
                        nc.gpsimd.partition_broadcast(rb[:], r1[:])
                        rbs.append(rb)
                    # attn-mean partial: (P0*r0 + P1*r1), bf16 out
                    for kt in range(NKT):
                        a0 = apool.tile([128, QB], BF16, tag="a0", name=f"a0_{b}_{qb}_{kt}")
                        nc.vector.tensor_tensor(a0[:], pts[(kt, 0)][:], rbs[0][:], OP.mult)
                        a1 = apool.tile([128, QB], BF16, tag="a1", name=f"a1_{b}_{qb}_{kt}")
                        nc.gpsimd.tensor_tensor(a1[:], pts[(kt, 1)][:], rbs[1][:], OP.mult)
                        nc.vector.tensor_tensor(a0[:], a0[:], a1[:], OP.add)
                        nc.sync.dma_start(attn_d[b, qb, kt], a0[:])
                    # dance pass 2: transpose normalized context back to [d, q]
                    for h2 in range(HPC):
                        for ch in range(QB // 128):
                            t2 = ps_d.tile([64, 128], F32R, tag="d", name=f"t2_{b}_{qb}_{h2}_{ch}")
                            nc.tensor.transpose(t2[:], cns[(h2, ch)][:], ident_r[:])
                            nc.vector.tensor_copy(
                                CT[:, h2, ch * 128:(ch + 1) * 128], t2[:]
                            )
                    # output projection partial for this (b, qb)
                    for tch in range(QB // 512):
                        for do in range(8):
                            po = ps_d.tile([128, 512], F32, tag="d", name=f"po_{b}_{qb}_{tch}_{do}")
                            for h2 in range(HPC):
                                nc.tensor.matmul(
                                    po[:], wo_sb[:, h2, do, :],
                                    CT[:, h2, tch * 512:(tch + 1) * 512],
                                    start=(h2 == 0), stop=(h2 == HPC - 1),
                                )
                            ob = opool.tile([128, 512], BF16, tag="ob", name=f"ob_{b}_{qb}_{tch}_{do}")
                            nc.any.tensor_copy(ob[:], po[:])
                            nc.sync.dma_start(
                                out1_d[do, (q0 + tch * 512) // 512], ob[:]
                            )

            blocks = [(b, qb) for b in range(B) for qb in range(S // QB)]
            prev = None
            for b, qb in blocks:
                head = attn_head(b, qb)
                if prev is not None:
                    attn_tail(*prev)
                prev = (b, qb, *head)
            attn_tail(*prev)
    nc.compile()
    return nc


def _get_nc():
    if "nc" not in _NC_CACHE:
        _NC_CACHE["nc"] = _build_core_program()
    return _NC_CACHE["nc"]


def kernel(x, Wq, bq, Wk, bk, Wv, bv, Wo, bo, ln_gamma, ln_beta):
    x = np.asarray(x, np.float32)
    Wq, bq = np.asarray(Wq, np.float32), np.asarray(bq, np.float32)
    Wk, bk = np.asarray(Wk, np.float32), np.asarray(bk, np.float32)
    Wv, bv = np.asarray(Wv, np.float32), np.asarray(bv, np.float32)
    Wo, bo = np.asarray(Wo, np.float32), np.asarray(bo, np.float32)
    g = np.asarray(ln_gamma, np.float32)
    be = np.asarray(ln_beta, np.float32)

    x2 = np.ascontiguousarray(x.reshape(T, D))

    in_maps = []
    for c in range(NCORES):
        sl = slice(128 * c, 128 * (c + 1))
        # fold LN gamma into W, LN beta into bias: proj = z @ (W*g).T + (W@be + b)
        wstk = np.stack(
            [
                (Wq * g[None, :])[sl, :],
                (Wk * g[None, :])[sl, :],
                (Wv * g[None, :])[sl, :],
            ],
            axis=0,
        )  # [3, 128, D]
        bstk = np.stack(
            [
                (Wq @ be + bq)[sl],
                (Wk @ be + bk)[sl],
                (Wv @ be + bv)[sl],
            ],
            axis=1,
        ).astype(np.float32)  # [128, 3]
        w_c = np.ascontiguousarray(
            wstk.transpose(2, 0, 1).astype(ml_dtypes.bfloat16)
        )  # [D, 3, 128]
        wo_c = np.ascontiguousarray(
            Wo[:, sl].T.reshape(HPC, DK, D).astype(np.float32)
        )  # [2, 64, D]
        in_maps.append({"x": x2, "wqkv": w_c, "bqkv": bstk, "wo": wo_c})

    nc = _get_nc()
    res = run_bass_kernel_spmd(nc, in_maps, core_ids=list(range(NCORES)))

    out1_acc = np.zeros((8, T // 512, 128, 512), np.float32)
    attn_acc = np.zeros((B, S // QB, NKT, 128, QB), np.float32)
    for r in res.results:
        out1_acc += r["out1_t"].astype(np.float32)
        attn_acc += r["attn_t"].astype(np.float32)

    out1 = out1_acc.transpose(0, 2, 1, 3).reshape(D, T).T  # [T, D]
    out1 = out1 + bo[None, :] + x2
    out = out1.reshape(B, S, D)
    # attn_acc[b, qb, kt, p, j] = A^T[k=kt*128+p, q=qb*QB+j]
    attn = np.ascontiguousarray(
        attn_acc.transpose(0, 1, 4, 2, 3).reshape(B, S, S)
    )
    return out, attn
